# revision 2
# baseline (speedup 1.0000x reference)
"""AWQ int4-quantized linear (nn_AWQLinear) as a Trainium2 Bass kernel.

Strategy: column-parallel over 8 NeuronCores (out_features sharded, x
replicated).  Per core:
  1. Dequantize the AWQ int4 weight shard on-device into SBUF-resident
     fp16 W [K=4096, N=1376]:  unpack nibbles with fused shift+and
     tensor_scalar ops, then W = w_int * A + B where A = scales and
     B = -zeros*scales are broadcast across partitions via 0-stride DMA.
     Weight columns are kept in nibble-major ("permuted") order so every
     DVE write is contiguous; the host un-permutes the output columns.
  2. Matmul out[M=8192, N] = x @ W on the PE: stationary = x^T tiles
     [128K, 128M], moving = W k-tiles [128K, <=512N], accumulating over
     32 k-tiles in PSUM.  x is transposed on the host once so all device
     DMAs are natural-layout (contiguous) reads.
"""

import numpy as np

import concourse.bacc as bacc
import concourse.tile as tile
from concourse import mybir
from concourse import bass_utils

F16 = mybir.dt.float16
F32 = mybir.dt.float32
I32 = mybir.dt.int32
ALU = mybir.AluOpType

IN_FEATURES = 4096
OUT_FEATURES = 11008
GROUP = 128
NCORES = 8
NSHARD = OUT_FEATURES // NCORES      # 1376
PACK = 8                             # int4s per int32
NPACK = NSHARD // PACK               # 172 packed columns per core
M_TOTAL = 4 * 2048                   # 8192
PSUM_N = 512


def build_nc(M=M_TOTAL, K=IN_FEATURES, n_shard=NSHARD, m_chunk=512,
             num_devices=NCORES):
    """Build the per-core SPMD Bass program."""
    KT = K // 128
    n_pack = n_shard // PACK
    assert M % m_chunk == 0 and m_chunk % 128 == 0

    n_chunks = []
    ns = 0
    while ns < n_shard:
        nl = min(PSUM_N, n_shard - ns)
        n_chunks.append((ns, nl))
        ns += nl

    nc = bacc.Bacc("TRN2", target_bir_lowering=False, debug=False,
                   num_devices=num_devices)
    xt_d = nc.dram_tensor("xt", [K, M], F16, kind="ExternalInput").ap()
    qw_d = nc.dram_tensor("qw", [K, n_pack], I32, kind="ExternalInput").ap()
    a_d = nc.dram_tensor("amat", [KT, n_shard], F16, kind="ExternalInput").ap()
    b_d = nc.dram_tensor("bmat", [KT, n_shard], F16, kind="ExternalInput").ap()
    out_d = nc.dram_tensor("out", [M, n_shard], F16, kind="ExternalOutput").ap()

    with tile.TileContext(nc) as tc:
        with (
            tc.tile_pool(name="wpool", bufs=1) as wpool,
            tc.tile_pool(name="qpool", bufs=2) as qpool,
            tc.tile_pool(name="abpool", bufs=2) as abpool,
            tc.tile_pool(name="xtpool", bufs=2) as xtpool,
            tc.tile_pool(name="pspool", bufs=4, space="PSUM") as pspool,
            tc.tile_pool(name="opool", bufs=3) as opool,
        ):
            # ---- Phase 1: dequantize the weight shard into SBUF ----
            w_tiles = []
            for k in range(KT):
                q = qpool.tile([128, n_pack], I32, tag="q")
                nc.sync.dma_start(q[:], qw_d[k * 128:(k + 1) * 128, :])
                arep = abpool.tile([128, n_shard], F16, tag="a")
                brep = abpool.tile([128, n_shard], F16, tag="b")
                nc.sync.dma_start(arep[:], a_d[k].partition_broadcast(128))
                nc.sync.dma_start(brep[:], b_d[k].partition_broadcast(128))

                wi = qpool.tile([128, n_shard], I32, tag="wi")
                for j in range(PACK):
                    # nibble j of each int32 -> int32 in [0, 16)
                    # (bitvec ops may not cast, so output stays int32)
                    nc.vector.tensor_scalar(
                        wi[:, j * n_pack:(j + 1) * n_pack], q[:], 4 * j, 15,
                        ALU.logical_shift_right, ALU.bitwise_and)
                w = wpool.tile([128, n_shard], F16, tag=f"w{k}")
                nc.vector.tensor_tensor(w[:], wi[:], arep[:], ALU.mult)
                nc.vector.tensor_tensor(w[:], w[:], brep[:], ALU.add)
                w_tiles.append(w)

            # ---- Phase 2: tiled matmul out = x @ W ----
            msubs = m_chunk // 128
            for mc in range(M // m_chunk):
                xts = []
                for k in range(KT):
                    xt = xtpool.tile([128, m_chunk], F16, tag=f"xt{k}")
                    nc.sync.dma_start(
                        xt[:], xt_d[k * 128:(k + 1) * 128,
                                    mc * m_chunk:(mc + 1) * m_chunk])
                    xts.append(xt)
                for ms in range(msubs):
                    m0 = mc * m_chunk + ms * 128
                    ob = opool.tile([128, n_shard], F16, tag="ob")
                    for (ns, nl) in n_chunks:
                        ps = pspool.tile([128, nl], F32, tag="ps")
                        for k in range(KT):
                            nc.tensor.matmul(
                                ps[:], xts[k][:, ms * 128:(ms + 1) * 128],
                                w_tiles[k][:, ns:ns + nl],
                                start=(k == 0), stop=(k == KT - 1))
                        nc.scalar.copy(ob[:, ns:ns + nl], ps[:])
                    nc.sync.dma_start(out_d[m0:m0 + 128, :], ob[:])
    return nc


def _perm_cols(a, n_pack):
    """[G, n] natural column order -> nibble-major device order."""
    g = a.shape[0]
    return np.ascontiguousarray(
        a.reshape(g, n_pack, PACK).transpose(0, 2, 1).reshape(g, -1))


def _unperm_cols(a, n_pack):
    """[M, n] nibble-major device order -> natural column order."""
    m = a.shape[0]
    return a.reshape(m, PACK, n_pack).transpose(0, 2, 1).reshape(m, -1)


_compiled_nc = None


def _get_compiled():
    global _compiled_nc
    if _compiled_nc is None:
        nc = build_nc()
        nc.compile()
        _compiled_nc = nc
    return _compiled_nc


def make_in_maps(x, qweight, qzeros, scales, n_cores=NCORES, n_shard=NSHARD):
    """Shard + marshal full inputs into per-core in_maps."""
    n_pack = n_shard // PACK
    m = int(np.prod(x.shape[:-1]))
    k = x.shape[-1]
    x2 = np.asarray(x, dtype=np.float16).reshape(m, k)
    xt = np.ascontiguousarray(x2.T)                      # [K, M] fp16

    shifts = np.arange(0, 32, 4, dtype=np.int32)
    z = ((np.asarray(qzeros)[:, :, None] >> shifts[None, None, :]) & 15)
    z = z.reshape(qzeros.shape[0], -1).astype(np.float32)  # [G, N] zeros
    s32 = np.asarray(scales).astype(np.float32)            # [G, N]
    a_full = s32
    b_full = -z * s32

    in_maps = []
    for c in range(n_cores):
        n0 = c * n_shard
        qw_c = np.ascontiguousarray(
            np.asarray(qweight)[:, c * n_pack:(c + 1) * n_pack])
        a_c = _perm_cols(a_full[:, n0:n0 + n_shard], n_pack).astype(np.float16)
        b_c = _perm_cols(b_full[:, n0:n0 + n_shard], n_pack).astype(np.float16)
        in_maps.append({"xt": xt, "qw": qw_c, "amat": a_c, "bmat": b_c})
    return in_maps


def kernel(x, qweight, qzeros, scales):
    x = np.asarray(x)
    nc = _get_compiled()
    in_maps = make_in_maps(x, qweight, qzeros, scales)
    res = bass_utils.run_bass_kernel_spmd(nc, in_maps,
                                          core_ids=list(range(NCORES)))
    outs = []
    for c in range(NCORES):
        op = res.results[c]["out"]                       # [M, NSHARD] permuted
        outs.append(_unperm_cols(op, NPACK))
    full = np.concatenate(outs, axis=1)                  # [M, OUT_FEATURES]
    return full.reshape(*x.shape[:-1], OUT_FEATURES).astype(np.float16)


# revision 4
# speedup vs baseline: 1.0555x; 1.0555x over previous
"""AWQ int4-quantized linear (nn_AWQLinear) as a Trainium2 Bass kernel.

Strategy: column-parallel over 8 NeuronCores (out_features sharded, x
replicated).  Per core:
  1. Dequantize the AWQ int4 weight shard on-device into SBUF-resident
     fp16 W [K=4096, N=1376]:  unpack nibbles with fused shift+and
     tensor_scalar ops, then W = w_int * A + B where A = scales and
     B = -zeros*scales are broadcast across partitions via 0-stride DMA.
     Weight columns are kept in nibble-major ("permuted") order so every
     DVE write is contiguous; the host un-permutes the output columns.
  2. Matmul out[M=8192, N] = x @ W on the PE: stationary = x^T tiles
     [128K, 128M], moving = W k-tiles [128K, <=512N], accumulating over
     32 k-tiles in PSUM.  x is transposed on the host once so all device
     DMAs are natural-layout (contiguous) reads.
"""

import numpy as np

import concourse.bacc as bacc
import concourse.tile as tile
from concourse import mybir
from concourse import bass_utils

F16 = mybir.dt.float16
F32 = mybir.dt.float32
I32 = mybir.dt.int32
ALU = mybir.AluOpType

IN_FEATURES = 4096
OUT_FEATURES = 11008
GROUP = 128
NCORES = 8
NSHARD = OUT_FEATURES // NCORES      # 1376
PACK = 8                             # int4s per int32
NPACK = NSHARD // PACK               # 172 packed columns per core
M_TOTAL = 4 * 2048                   # 8192
PSUM_N = 512


def build_nc(M=M_TOTAL, K=IN_FEATURES, n_shard=NSHARD, m_chunk=512,
             num_devices=NCORES, repeat=1):
    """Build the per-core SPMD Bass program."""
    KT = K // 128
    n_pack = n_shard // PACK
    assert M % m_chunk == 0 and m_chunk % 128 == 0

    n_chunks = []
    ns = 0
    while ns < n_shard:
        nl = min(PSUM_N, n_shard - ns)
        n_chunks.append((ns, nl))
        ns += nl

    nc = bacc.Bacc("TRN2", target_bir_lowering=False, debug=False,
                   num_devices=num_devices)
    xt_d = nc.dram_tensor("xt", [K, M], F16, kind="ExternalInput").ap()
    qw_d = nc.dram_tensor("qw", [K, n_pack], I32, kind="ExternalInput").ap()
    a_d = nc.dram_tensor("amat", [KT, n_shard], F16, kind="ExternalInput").ap()
    b_d = nc.dram_tensor("bmat", [KT, n_shard], F16, kind="ExternalInput").ap()
    out_d = nc.dram_tensor("out", [M, n_shard], F16, kind="ExternalOutput").ap()

    with tile.TileContext(nc) as tc:
        with (
            tc.tile_pool(name="wpool", bufs=1) as wpool,
            tc.tile_pool(name="qpool", bufs=2) as qpool,
            tc.tile_pool(name="abpool", bufs=2) as abpool,
            tc.tile_pool(name="xtpool", bufs=2) as xtpool,
            tc.tile_pool(name="pspool", bufs=4, space="PSUM") as pspool,
            tc.tile_pool(name="opool", bufs=3) as opool,
        ):
          for _rep in range(repeat):
            # ---- Phase 1: dequantize the weight shard into SBUF ----
            w_tiles = []
            for k in range(KT):
                q = qpool.tile([128, n_pack], I32, tag="q")
                nc.sync.dma_start(q[:], qw_d[k * 128:(k + 1) * 128, :])
                arep = abpool.tile([128, n_shard], F16, tag="a")
                brep = abpool.tile([128, n_shard], F16, tag="b")
                nc.sync.dma_start(arep[:], a_d[k].partition_broadcast(128))
                nc.sync.dma_start(brep[:], b_d[k].partition_broadcast(128))

                wi = qpool.tile([128, n_shard], I32, tag="wi")
                for j in range(PACK):
                    # nibble j of each int32 -> int32 in [0, 16)
                    # (bitvec ops may not cast, so output stays int32)
                    nc.vector.tensor_scalar(
                        wi[:, j * n_pack:(j + 1) * n_pack], q[:], 4 * j, 15,
                        ALU.logical_shift_right, ALU.bitwise_and)
                w = wpool.tile([128, n_shard], F16, tag=f"w{k}")
                nc.vector.tensor_tensor(w[:], wi[:], arep[:], ALU.mult)
                nc.vector.tensor_tensor(w[:], w[:], brep[:], ALU.add)
                w_tiles.append(w)

            # ---- Phase 2: tiled matmul out = x @ W ----
            msubs = m_chunk // 128
            for mc in range(M // m_chunk):
                xts = []
                for k in range(KT):
                    xt = xtpool.tile([128, m_chunk], F16, tag=f"xt{k}")
                    nc.sync.dma_start(
                        xt[:], xt_d[k * 128:(k + 1) * 128,
                                    mc * m_chunk:(mc + 1) * m_chunk])
                    xts.append(xt)
                for ms in range(msubs):
                    m0 = mc * m_chunk + ms * 128
                    ob = opool.tile([128, n_shard], F16, tag="ob")
                    for (ns, nl) in n_chunks:
                        ps = pspool.tile([128, nl], F32, tag="ps")
                        for k in range(KT):
                            nc.tensor.matmul(
                                ps[:], xts[k][:, ms * 128:(ms + 1) * 128],
                                w_tiles[k][:, ns:ns + nl],
                                start=(k == 0), stop=(k == KT - 1))
                        nc.scalar.copy(ob[:, ns:ns + nl], ps[:])
                    nc.sync.dma_start(out_d[m0:m0 + 128, :], ob[:])
    return nc


def _perm_cols(a, n_pack):
    """[G, n] natural column order -> nibble-major device order."""
    g = a.shape[0]
    return np.ascontiguousarray(
        a.reshape(g, n_pack, PACK).transpose(0, 2, 1).reshape(g, -1))


def _unperm_cols(a, n_pack):
    """[M, n] nibble-major device order -> natural column order."""
    m = a.shape[0]
    return a.reshape(m, PACK, n_pack).transpose(0, 2, 1).reshape(m, -1)


_compiled_nc = None


def _get_compiled():
    global _compiled_nc
    if _compiled_nc is None:
        nc = build_nc()
        nc.compile()
        _compiled_nc = nc
    return _compiled_nc


def make_in_maps(x, qweight, qzeros, scales, n_cores=NCORES, n_shard=NSHARD):
    """Shard + marshal full inputs into per-core in_maps."""
    n_pack = n_shard // PACK
    m = int(np.prod(x.shape[:-1]))
    k = x.shape[-1]
    x2 = np.asarray(x, dtype=np.float16).reshape(m, k)
    xt = np.ascontiguousarray(x2.T)                      # [K, M] fp16

    shifts = np.arange(0, 32, 4, dtype=np.int32)
    z = ((np.asarray(qzeros)[:, :, None] >> shifts[None, None, :]) & 15)
    z = z.reshape(qzeros.shape[0], -1).astype(np.float32)  # [G, N] zeros
    s32 = np.asarray(scales).astype(np.float32)            # [G, N]
    a_full = s32
    b_full = -z * s32

    in_maps = []
    for c in range(n_cores):
        n0 = c * n_shard
        qw_c = np.ascontiguousarray(
            np.asarray(qweight)[:, c * n_pack:(c + 1) * n_pack])
        a_c = _perm_cols(a_full[:, n0:n0 + n_shard], n_pack).astype(np.float16)
        b_c = _perm_cols(b_full[:, n0:n0 + n_shard], n_pack).astype(np.float16)
        in_maps.append({"xt": xt, "qw": qw_c, "amat": a_c, "bmat": b_c})
    return in_maps


def kernel(x, qweight, qzeros, scales):
    x = np.asarray(x)
    nc = _get_compiled()
    in_maps = make_in_maps(x, qweight, qzeros, scales)
    res = bass_utils.run_bass_kernel_spmd(nc, in_maps,
                                          core_ids=list(range(NCORES)))
    outs = []
    for c in range(NCORES):
        op = res.results[c]["out"]                       # [M, NSHARD] permuted
        outs.append(_unperm_cols(op, NPACK))
    full = np.concatenate(outs, axis=1)                  # [M, OUT_FEATURES]
    return full.reshape(*x.shape[:-1], OUT_FEATURES).astype(np.float16)


# revision 30
# speedup vs baseline: 1.3023x; 1.2338x over previous
"""AWQ int4-quantized linear (nn_AWQLinear) as a Trainium2 Bass kernel.

Strategy: column-parallel over 8 NeuronCores (out_features sharded, x
replicated).  Per core:
  1. Dequantize the AWQ int4 weight shard on-device into SBUF-resident
     fp16 W [K=4096, N=1376]:  unpack nibbles with fused shift+and
     tensor_scalar ops, then W = w_int * A + B where A = scales and
     B = -zeros*scales are broadcast across partitions via 0-stride DMA.
     Weight columns are kept in nibble-major ("permuted") order so every
     DVE write is contiguous; the host un-permutes the output columns.
  2. Matmul out[M=8192, N] = x @ W on the PE: stationary = x^T tiles
     [128K, 128M], moving = W k-tiles [128K, <=512N], accumulating over
     32 k-tiles in PSUM.  x is transposed on the host once so all device
     DMAs are natural-layout (contiguous) reads.
"""

import numpy as np

import concourse.bacc as bacc
import concourse.tile as tile
from concourse import mybir
from concourse import bass_utils

F16 = mybir.dt.float16
F32 = mybir.dt.float32
I32 = mybir.dt.int32
ALU = mybir.AluOpType

IN_FEATURES = 4096
OUT_FEATURES = 11008
GROUP = 128
NCORES = 8
NSHARD = OUT_FEATURES // NCORES      # 1376
PACK = 8                             # int4s per int32
NPACK = NSHARD // PACK               # 172 packed columns per core
M_TOTAL = 4 * 2048                   # 8192
PSUM_N = 512


def build_nc(M=M_TOTAL, K=IN_FEATURES, n_shard=NSHARD, m_chunk=512,
             num_devices=NCORES, repeat=1, mode="full", korder=False,
             u16=False, chunk_plan=None, split_w=False):
    """Build the per-core SPMD Bass program.

    mode: "full" (dequant + matmul), "mm_only" (dequantized W supplied as
    input; timing experiment), "deq_only" (no matmul; timing experiment).
    korder: k-outer matmul loop (stationary tile reused across n-chunks).
    u16: unpack from uint16 view of qweight (half the unpack op count).
    chunk_plan: list of moving-operand widths per k-tile; sum >= n_shard.
      Widths beyond n_shard are padding (junk columns, never stored out).
    """
    KT = K // 128
    n_pack = n_shard // PACK
    assert M % m_chunk == 0 and m_chunk % 128 == 0

    n_chunks = []
    ns = 0
    for nl in (chunk_plan or []):
        n_chunks.append((ns, nl))
        ns += nl
    if not n_chunks:
        while ns < n_shard:
            nl = min(PSUM_N, n_shard - ns)
            n_chunks.append((ns, nl))
            ns += nl
    ob_w = sum(nl for _, nl in n_chunks)
    assert ob_w >= n_shard

    nc = bacc.Bacc("TRN2", target_bir_lowering=False, debug=False,
                   num_devices=num_devices)
    xt_d = nc.dram_tensor("xt", [K, M], F16, kind="ExternalInput").ap()
    if mode == "mm_only":
        wdeq_d = nc.dram_tensor("wdeq", [K, n_shard], F16,
                                kind="ExternalInput").ap()
    else:
        if u16:
            qw_d = nc.dram_tensor("qw", [K, 2 * n_pack], mybir.dt.uint16,
                                  kind="ExternalInput").ap()
        else:
            qw_d = nc.dram_tensor("qw", [K, n_pack], I32,
                                  kind="ExternalInput").ap()
        a_d = nc.dram_tensor("amat", [KT, n_shard], F16,
                             kind="ExternalInput").ap()
        b_d = nc.dram_tensor("bmat", [KT, n_shard], F16,
                             kind="ExternalInput").ap()
    out_d = nc.dram_tensor("out", [M, n_shard], F16, kind="ExternalOutput").ap()

    with tile.TileContext(nc) as tc:
        with (
            tc.tile_pool(name="wpool", bufs=1) as wpool,
            tc.tile_pool(name="qpool", bufs=2) as qpool,
            tc.tile_pool(name="abpool", bufs=2) as abpool,
            tc.tile_pool(name="xtpool", bufs=2) as xtpool,
            tc.tile_pool(name="pspool", bufs=6, space="PSUM") as pspool,
            tc.tile_pool(name="opool", bufs=3) as opool,
        ):
          for _rep in range(repeat):
            # ---- Phase 1: dequantize the weight shard into SBUF ----
            w_tiles = []
            for k in range(KT):
                if mode == "mm_only":
                    w = wpool.tile([128, n_shard], F16, tag=f"w{k}")
                    nc.sync.dma_start(w[:], wdeq_d[k * 128:(k + 1) * 128, :])
                    w_tiles.append(w)
                    continue
                if u16:
                    q = qpool.tile([128, 2 * n_pack], mybir.dt.uint16, tag="q")
                else:
                    q = qpool.tile([128, n_pack], I32, tag="q")
                nc.sync.dma_start(q[:], qw_d[k * 128:(k + 1) * 128, :])
                arep = abpool.tile([128, n_shard], F16, tag="a")
                brep = abpool.tile([128, n_shard], F16, tag="b")
                nc.sync.dma_start(arep[:], a_d[k].partition_broadcast(128))
                nc.sync.dma_start(brep[:], b_d[k].partition_broadcast(128))

                # bitvec ops may not cast, so unpack keeps the input dtype
                if u16:
                    wi = qpool.tile([128, n_shard], mybir.dt.uint16, tag="wi")
                    cw = 2 * n_pack
                    for j in range(PACK // 2):
                        nc.vector.tensor_scalar(
                            wi[:, j * cw:(j + 1) * cw], q[:], 4 * j, 15,
                            ALU.logical_shift_right, ALU.bitwise_and)
                else:
                    wi = qpool.tile([128, n_shard], I32, tag="wi")
                    for j in range(PACK):
                        nc.vector.tensor_scalar(
                            wi[:, j * n_pack:(j + 1) * n_pack], q[:], 4 * j, 15,
                            ALU.logical_shift_right, ALU.bitwise_and)
                if split_w:
                    # per-chunk whole tiles: the PE moving operand must not
                    # be a slice of a wide tile (HW streams those ~2x slower)
                    wrow = []
                    for ci, (ns2, nl) in enumerate(n_chunks):
                        wt = wpool.tile([128, nl], F16, tag=f"w{k}_{ci}")
                        real = min(nl, n_shard - ns2)
                        if real < nl:
                            nc.vector.memset(wt[:, real:nl], 0.0)
                        nc.vector.tensor_tensor(
                            wt[:, 0:real], wi[:, ns2:ns2 + real],
                            arep[:, ns2:ns2 + real], ALU.mult)
                        nc.vector.tensor_tensor(
                            wt[:, 0:real], wt[:, 0:real],
                            brep[:, ns2:ns2 + real], ALU.add)
                        wrow.append(wt)
                    w_tiles.append(wrow)
                else:
                    w = wpool.tile([128, n_shard], F16, tag=f"w{k}")
                    nc.vector.tensor_tensor(w[:], wi[:], arep[:], ALU.mult)
                    nc.vector.tensor_tensor(w[:], w[:], brep[:], ALU.add)
                    w_tiles.append(w)

            if mode == "deq_only":
                # flush one W tile to DRAM so the work isn't dead-code
                nc.sync.dma_start(out_d[0:128, :], w_tiles[-1][:])
                continue
            # ---- Phase 2: tiled matmul out = x @ W ----
            msubs = m_chunk // 128
            xts_once = None
            if mode == "mm_xonce":     # timing experiment: no steady-state DMA
                xts_once = []
                for k in range(KT):
                    xt = xtpool.tile([128, m_chunk], F16, tag=f"xt{k}")
                    nc.sync.dma_start(xt[:], xt_d[k * 128:(k + 1) * 128,
                                                  0:m_chunk])
                    xts_once.append(xt)
            for mc in range(M // m_chunk):
                if xts_once is not None:
                    xts = xts_once
                else:
                    xts = []
                    for k in range(KT):
                        xt = xtpool.tile([128, m_chunk], F16, tag=f"xt{k}")
                        nc.sync.dma_start(
                            xt[:], xt_d[k * 128:(k + 1) * 128,
                                        mc * m_chunk:(mc + 1) * m_chunk])
                        xts.append(xt)
                def mov(k, ci, ns, nl):
                    if split_w:
                        return w_tiles[k][ci][:]
                    return w_tiles[k][:, ns:ns + nl]

                for ms in range(msubs):
                    m0 = mc * m_chunk + ms * 128
                    # one PSUM group -> one narrow whole-tile ob -> one out
                    # DMA slice.  (A wide shared ob written by several sliced
                    # ACT copies drops the PE stream to ~1 col/cycle.)
                    for ci, (ns, nl) in enumerate(n_chunks):
                        real = min(nl, n_shard - ns)
                        ps = pspool.tile([128, nl], F32, tag="ps")
                        for k in range(KT):
                            nc.tensor.matmul(
                                ps[:], xts[k][:, ms * 128:(ms + 1) * 128],
                                mov(k, ci, ns, nl),
                                start=(k == 0), stop=(k == KT - 1))
                        obc = opool.tile([128, nl], F16, tag=f"ob{ci}")
                        nc.scalar.copy(obc[:], ps[:])
                        nc.sync.dma_start(out_d[m0:m0 + 128, ns:ns + real],
                                          obc[:, 0:real])
    return nc


def _n_of_p(n_shard, u16):
    """Map device (permuted) column p -> natural column n within a shard."""
    n_pack = n_shard // PACK
    p = np.arange(n_shard)
    if u16:
        # device unpacks uint16 halves: p = j2*(2*n_pack) + 2*c + half,
        # natural n = c*8 + 4*half + j2  (j2 = nibble within the uint16)
        j2 = p // (2 * n_pack)
        r = p % (2 * n_pack)
        c, half = r // 2, r % 2
        return c * 8 + 4 * half + j2
    # int32 path: p = j*n_pack + c, natural n = c*8 + j
    j, c = p // n_pack, p % n_pack
    return c * 8 + j


def _perm_cols(a, n_pack, u16=False):
    """[G, n] natural column order -> device (nibble-major) order."""
    return np.ascontiguousarray(a[:, _n_of_p(a.shape[1], u16)])


def _unperm_cols(a, n_pack, u16=False):
    """[M, n] device order -> natural column order."""
    nop = _n_of_p(a.shape[1], u16)
    inv = np.empty_like(nop)
    inv[nop] = np.arange(len(nop))
    return a[:, inv]


U16 = True          # unpack from uint16 view of qweight (faster dequant)
SPLIT_W = True      # per-chunk whole W tiles (fast PE moving-operand stream)

_compiled_nc = None


def _get_compiled():
    global _compiled_nc
    if _compiled_nc is None:
        nc = build_nc(u16=U16, split_w=SPLIT_W)
        nc.compile()
        _compiled_nc = nc
    return _compiled_nc


def make_in_maps(x, qweight, qzeros, scales, n_cores=NCORES, n_shard=NSHARD,
                 u16=False):
    """Shard + marshal full inputs into per-core in_maps."""
    n_pack = n_shard // PACK
    m = int(np.prod(x.shape[:-1]))
    k = x.shape[-1]
    x2 = np.asarray(x, dtype=np.float16).reshape(m, k)
    xt = np.ascontiguousarray(x2.T)                      # [K, M] fp16

    shifts = np.arange(0, 32, 4, dtype=np.int32)
    z = ((np.asarray(qzeros)[:, :, None] >> shifts[None, None, :]) & 15)
    z = z.reshape(qzeros.shape[0], -1).astype(np.float32)  # [G, N] zeros
    s32 = np.asarray(scales).astype(np.float32)            # [G, N]
    a_full = s32
    b_full = -z * s32

    in_maps = []
    for c in range(n_cores):
        n0 = c * n_shard
        qw_c = np.ascontiguousarray(
            np.asarray(qweight)[:, c * n_pack:(c + 1) * n_pack])
        if u16:
            qw_c = qw_c.view(np.uint16)                  # [K, 2*n_pack]
        a_c = _perm_cols(a_full[:, n0:n0 + n_shard], n_pack,
                         u16).astype(np.float16)
        b_c = _perm_cols(b_full[:, n0:n0 + n_shard], n_pack,
                         u16).astype(np.float16)
        in_maps.append({"xt": xt, "qw": qw_c, "amat": a_c, "bmat": b_c})
    return in_maps


def kernel(x, qweight, qzeros, scales):
    x = np.asarray(x)
    nc = _get_compiled()
    in_maps = make_in_maps(x, qweight, qzeros, scales, u16=U16)
    res = bass_utils.run_bass_kernel_spmd(nc, in_maps,
                                          core_ids=list(range(NCORES)))
    outs = []
    for c in range(NCORES):
        op = res.results[c]["out"]                       # [M, NSHARD] permuted
        outs.append(_unperm_cols(op, NPACK, u16=U16))
    full = np.concatenate(outs, axis=1)                  # [M, OUT_FEATURES]
    return full.reshape(*x.shape[:-1], OUT_FEATURES).astype(np.float16)
